# revision 35
# baseline (speedup 1.0000x reference)
"""Distributed Bass kernel: RMSNorm + multi-head attention + out-proj on 8 TRN2 cores.

Sharding: head x batch tensor parallel. Core c owns batch c//4 and heads
[4*(c%4), 4*(c%4)+4) for the full 2048-token sequence. Each core RMSNorms the
whole batch, projects Q/K/V for only its 4 heads (w_qkv column shard), runs
full attention for those heads, and computes a partial output projection
(w_out row shard). A single bf16 ReduceScatter per token-half sums the 4
partials of each batch group and scatters 512 rows back to each core - the
only collective in the kernel (the baseline's 8 serialized K/V AllGathers
cost ~330us on the collective cores).

Attention pipeline per (head, 128-query tile): q-major sim on the PE
(fp16, x8 scale folded into w_q), exact row-max via DVE+Pool psum reduces,
one ScalarE exp pass (bias = -rowmax), DMA-xbar transpose of the bf16 attn
tile into keys-major layout, then a full-PE AV matmul (lhsT = attnT tile,
128x128 stationary; moving operand = [v | ones], 65 columns) whose extra
ones-column yields the softmax denominator for free. Normalization happens
on the tiny [128, 64] AV output, not the [128, 2048] attn matrix.
"""

import sys

sys.path.insert(0, "/opt/trn_rl_repo")

import numpy as np
import ml_dtypes

import concourse.bass as bass
import concourse.mybir as mybir
import concourse.tile as tile
from concourse import bacc
from concourse.bass_utils import run_bass_kernel_spmd
from concourse.masks import make_identity

F32 = mybir.dt.float32
F16 = mybir.dt.float16
BF16 = mybir.dt.bfloat16
AF = mybir.ActivationFunctionType
ALU = mybir.AluOpType

B, N, D = 2, 2048, 1024
H, DH = 16, 64
EPS = 1e-5
NC_TOTAL = 8
HPC = 4                 # heads per core
GROUP = 4               # cores per batch (reduce-scatter group)
NT = N // 128           # 16 token tiles
QT = NT                 # query tiles
KC = NT                 # key chunks of 128
DC = D // 128           # 8 contraction chunks
WQKV_COLS = 3 * HPC * DH  # 768


def build_graph():
    nc = bacc.Bacc(name="attn8")
    x_d = nc.dram_tensor("x", [N, D], F16, kind="ExternalInput")
    w_d = nc.dram_tensor("w_qkv", [D, WQKV_COLS], F16, kind="ExternalInput")
    wout_d = nc.dram_tensor("w_out", [HPC * DH, D], BF16, kind="ExternalInput")
    outp_d = nc.dram_tensor("outp", [N, D], BF16, kind="Internal")
    rsout_d = nc.dram_tensor("rsout", [N // GROUP, D], BF16, kind="Internal")
    out_d = nc.dram_tensor("out", [N // GROUP, D], BF16,
                           kind="ExternalOutput")  # [512, 1024]

    rg = [list(range(GROUP)), list(range(GROUP, 2 * GROUP))]

    with tile.TileContext(nc) as tc:
        with (
            tc.tile_pool(name="const", bufs=1) as constp,
            tc.tile_pool(name="xload", bufs=6) as xp,
            tc.tile_pool(name="xnorm", bufs=NT) as xnp,
            tc.tile_pool(name="xnT", bufs=DC) as xntp,
            tc.tile_pool(name="wqkv", bufs=DC) as wp,
            tc.tile_pool(name="wout", bufs=2) as woutp,
            tc.tile_pool(name="kq", bufs=2) as kqp,
            tc.tile_pool(name="vx", bufs=HPC) as vxp,
            tc.tile_pool(name="stats", bufs=8) as statsp,
            tc.tile_pool(name="scr", bufs=3) as scrp,
            tc.tile_pool(name="attn", bufs=5) as attnp,
            tc.tile_pool(name="attnT", bufs=9) as attntp,
            tc.tile_pool(name="aout", bufs=2 * QT) as aoutp,
            tc.tile_pool(name="aoutT", bufs=2) as aouttp,
            tc.tile_pool(name="osb", bufs=3) as osbp,
            tc.tile_pool(name="ps_a", bufs=7, space="PSUM") as psa,
            tc.tile_pool(name="ps_b", bufs=1, space="PSUM") as psb,
        ):
            identf = constp.tile([128, 128], F16, name="identf")
            make_identity(nc, identf[:])
            identb = constp.tile([128, 128], BF16, name="identb")
            make_identity(nc, identb[:])
            epsb = constp.tile([128, 1], F32, name="epsb")
            nc.any.memset(epsb[:], EPS)

            # ---------------- DMA loads ----------------
            xt = []
            for t in range(NT):
                xl = xp.tile([128, D], F16, name=f"x{t}", tag="x")
                nc.sync.dma_start(xl[:], x_d[t * 128:(t + 1) * 128, :])
                xt.append(xl)
            w_sb = []
            for dc in range(DC):
                w = wp.tile([128, WQKV_COLS], F16, name=f"w{dc}", tag="w")
                nc.sync.dma_start(w[:], w_d[dc * 128:(dc + 1) * 128, :])
                w_sb.append(w)
            wout_sb = []
            for i in range(2):
                w = woutp.tile([128, D], BF16, name=f"wo{i}", tag="wo")
                nc.sync.dma_start(w[:], wout_d[i * 128:(i + 1) * 128, :])
                wout_sb.append(w)

            # ---- RMSNorm + transpose + projections, half-interleaved so the
            # PE starts transposing/projecting while the second token half is
            # still normalizing.
            xn = [None] * NT
            xnT = []
            for dc in range(DC):
                xT = xntp.tile([128, N], F16, name=f"xnT{dc}", tag="xnT")
                xnT.append(xT)

            def norm_tile(t):
                scr = scrp.tile([128, D], F16, name=f"scr{t}", tag="scr")
                ssq = statsp.tile([128, 1], F32, name=f"ssq{t}", tag="ssq")
                nc.scalar.activation(scr[:], xt[t][:], AF.Square,
                                     accum_out=ssq[:])
                std = statsp.tile([128, 1], F32, name=f"std{t}", tag="ssq")
                nc.scalar.activation(std[:], ssq[:], AF.Sqrt, scale=1.0 / D,
                                     bias=epsb[:])
                rinv = statsp.tile([128, 1], F32, name=f"ri{t}", tag="ssq")
                nc.vector.reciprocal(rinv[:], std[:])
                x2 = xnp.tile([128, D], F16, name=f"xn{t}", tag="xn")
                nc.vector.tensor_scalar_mul(x2[:], xt[t][:], rinv[:])
                xn[t] = x2

            def xnT_half(dc, half):
                for sub in range(2):
                    tp = psa.tile([128, 512], F16, name=f"tp{dc}{half}{sub}",
                                  tag="sim")
                    for j in range(4):
                        t = half * 8 + sub * 4 + j
                        nc.tensor.transpose(
                            tp[:, j * 128:(j + 1) * 128],
                            xn[t][:, dc * 128:(dc + 1) * 128],
                            identf[:])
                    col = half * 1024 + sub * 512
                    nc.vector.tensor_copy(
                        xnT[dc][:, col:col + 512], tp[:])

            # kT/qT feature-major pair tiles [128 feats(2 heads), 2048 tok]
            kTp = [kqp.tile([128, N], F16, name=f"kT{i}", tag="kT", bufs=2)
                   for i in range(2)]
            qTp = [kqp.tile([128, N], F16, name=f"qT{i}", tag="qT", bufs=2)
                   for i in range(2)]

            def proj_half(pt, col0, i, half):
                for tc2 in range(2):
                    tcol = half * 1024 + tc2 * 512
                    ps = psa.tile([128, 512], F32, name=f"pp{col0}{i}{tcol}",
                                  tag="sim")
                    for dc in range(DC):
                        nc.tensor.matmul(
                            ps[:],
                            w_sb[dc][:, col0 + i * 128:col0 + (i + 1) * 128],
                            xnT[dc][:, tcol:tcol + 512],
                            start=(dc == 0), stop=(dc == DC - 1))
                    nc.vector.tensor_copy(pt[:, tcol:tcol + 512], ps[:])

            # v token-major, per head [128 k-part, 16 kc * 65] bf16 with a
            # ones column at slot 64 of each kc block (softmax denominator).
            vx = []
            for h in range(HPC):
                v = vxp.tile([128, KC * 65], BF16, name=f"vx{h}", tag="vx")
                nc.any.memset(
                    v[:].rearrange("p (kc c) -> p kc c", c=65)[:, :, 64:65],
                    1.0)
                vx.append(v)

            def v_proj(t):
                ps = psa.tile([128, 512], F32, name=f"pv{t}", tag="sim")
                for dc in range(DC):
                    nc.tensor.matmul(
                        ps[:, 0:HPC * DH],
                        xnT[dc][:, t * 128:(t + 1) * 128],
                        w_sb[dc][:, 2 * HPC * DH:3 * HPC * DH],
                        start=(dc == 0), stop=(dc == DC - 1))
                for h in range(HPC):
                    nc.vector.tensor_copy(
                        vx[h][:, t * 65:t * 65 + 64],
                        ps[:, h * 64:(h + 1) * 64])

            for t in range(8):
                norm_tile(t)
            for dc in range(DC):
                xnT_half(dc, 0)
            proj_half(kTp[0], HPC * DH, 0, 0)
            proj_half(qTp[0], 0, 0, 0)
            for t in range(8, NT):
                norm_tile(t)
            for dc in range(DC):
                xnT_half(dc, 1)
            proj_half(kTp[0], HPC * DH, 0, 1)
            proj_half(qTp[0], 0, 0, 1)
            for t in range(NT):
                v_proj(t)
            pre_work = [
                lambda h=half: proj_half(kTp[1], HPC * DH, 1, h)
                for half in range(2)
            ] + [
                lambda h=half: proj_half(qTp[1], 0, 1, h)
                for half in range(2)
            ]

            # ---------------- attention (software-pipelined) ----------------
            # Unit = (head, 128-query tile). front() runs sim -> max -> exp ->
            # DMA transpose; back() runs AV + normalize. back(u) is emitted
            # LAG units after front(u) so the in-order PE never stalls on the
            # cross-engine max/exp/transpose chain.
            aout_tiles = {}  # (hp, qt) -> [128 q, 128 f] bf16 pair tile
            aoutT = {0: None, 1: None}
            unit_state = {}

            def front(h, qt):
                i, row = h // 2, (h % 2) * 64
                sims = []
                sa = statsp.tile([128, 4], F32, name=f"sa{h}{qt}", tag="sa")
                for qtr in range(4):
                    ps = psa.tile([128, 512], F32, name=f"s{h}{qt}{qtr}",
                                  tag="sim")
                    nc.tensor.matmul(
                        ps[:],
                        qTp[i][row:row + 64, qt * 128:(qt + 1) * 128],
                        kTp[i][row:row + 64, qtr * 512:(qtr + 1) * 512],
                        start=True, stop=True)
                    nc.vector.tensor_reduce(
                        sa[:, qtr:qtr + 1], ps[:],
                        axis=mybir.AxisListType.X, op=ALU.max)
                    sims.append(ps)
                negm = statsp.tile([128, 1], F32, name=f"nm{h}{qt}", tag="nm")
                nc.vector.tensor_reduce(negm[:], sa[:],
                                        axis=mybir.AxisListType.X,
                                        op=ALU.max, negate=True)
                at = attnp.tile([128, N], BF16, name=f"at{h}{qt}", tag="at")
                for qtr in range(4):
                    nc.scalar.activation(
                        at[:, qtr * 512:(qtr + 1) * 512],
                        sims[qtr][:], AF.Exp, bias=negm[:])
                atT = attntp.tile([128, KC * 128], BF16, name=f"atT{h}{qt}",
                                  tag="atT")
                nc.sync.dma_start_transpose(
                    atT[:].rearrange("p (kc q) -> p kc q", q=128), at[:])
                unit_state[(h, qt)] = atT

            av_rot = {"tile": None, "n": 0}

            def back_av(h, qt):
                atT = unit_state.pop((h, qt))
                if av_rot["n"] == 0:
                    av_rot["tile"] = psb.tile([128, 512], F32,
                                              name=f"av{h}{qt}", tag="av")
                j = av_rot["n"]
                av_rot["n"] = (j + 1) % 7
                av = av_rot["tile"][:, j * 65:(j + 1) * 65]
                atT3 = atT[:].rearrange("p (kc q) -> p kc q", q=128)
                for kc in range(KC):
                    nc.tensor.matmul(
                        av,
                        atT3[:, kc, :],
                        vx[h][:, kc * 65:(kc + 1) * 65],
                        start=(kc == 0), stop=(kc == KC - 1))
                unit_state[("av", h, qt)] = av

            def back_cp(h, qt):
                av = unit_state.pop(("av", h, qt))
                avs = statsp.tile([128, 65], F32, name=f"avs{h}{qt}",
                                  tag="avs", bufs=4)
                nc.scalar.copy(avs[:], av)
                unit_state[("avs", h, qt)] = avs

            def back_norm(h, qt):
                avs = unit_state.pop(("avs", h, qt))
                rs = statsp.tile([128, 1], F32, name=f"rs{h}{qt}", tag="rs")
                nc.vector.reciprocal(rs[:], avs[:, 64:65])
                hp = h // 2
                if (hp, qt) not in aout_tiles:
                    aout_tiles[(hp, qt)] = aoutp.tile(
                        [128, 128], BF16, name=f"ao{hp}{qt}", tag="ao")
                # SBUF->SBUF per-q scale on the idle Pool engine
                nc.gpsimd.tensor_scalar_mul(
                    aout_tiles[(hp, qt)][:, (h % 2) * 64:(h % 2) * 64 + 64],
                    avs[:, 0:64], rs[:])

            def aout_transpose(hp, quarter):
                # transpose this quarter's aout pair tiles into aoutT[hp]
                if aoutT[hp] is None:
                    aoutT[hp] = aouttp.tile([128, N], BF16, name=f"aoT{hp}",
                                            tag="aT")
                aT = aoutT[hp]
                tp = psa.tile([128, 512], BF16, name=f"tpa{hp}{quarter}",
                              tag="sim")
                for j in range(4):
                    qt = quarter * 4 + j
                    nc.tensor.transpose(
                        tp[:, j * 128:(j + 1) * 128],
                        aout_tiles[(hp, qt)][:], identb[:])
                col = quarter * 512
                nc.vector.tensor_copy(aT[:, col:col + 512], tp[:])

            oproj_tiles = {}

            def outproj_oc(qt, oc):
                if qt not in oproj_tiles:
                    oproj_tiles[qt] = osbp.tile([128, D], BF16, name=f"o{qt}",
                                                tag="o")
                ot = oproj_tiles[qt]
                ps = psa.tile([128, 512], F32, name=f"po{qt}{oc}", tag="sim")
                for hp in range(2):
                    nc.tensor.matmul(
                        ps[:],
                        aoutT[hp][:, qt * 128:(qt + 1) * 128],
                        wout_sb[hp][:, oc * 512:(oc + 1) * 512],
                        start=(hp == 0), stop=(hp == 1))
                if oc == 0:
                    nc.vector.tensor_copy(ot[:, 0:512], ps[:])
                else:
                    nc.scalar.copy(ot[:, 512:1024], ps[:])
                if oc == 1:
                    # Pool SWDGE queue so output DMAs never block the attn
                    # transposes queued on SP
                    nc.gpsimd.dma_start(outp_d[qt * 128:(qt + 1) * 128, :],
                                        ot[:])

            def reduce_scatter(half):
                import os as _os
                if _os.environ.get("KERNEL_FAKE_COMM") == "1":
                    nc.sync.dma_start(
                        rsout_d[half * 256:(half + 1) * 256, :],
                        outp_d[half * 1024:half * 1024 + 256, :])
                else:
                    nc.gpsimd.collective_compute(
                        "ReduceScatter", ALU.add, replica_groups=rg,
                        ins=[outp_d[half * 1024:(half + 1) * 1024, :].opt()],
                        outs=[rsout_d[half * 256:(half + 1) * 256, :].opt()])
                nc.sync.dma_start(out_d[half * 256:(half + 1) * 256, :],
                                  rsout_d[half * 256:(half + 1) * 256, :])

            # Software pipeline: AV trails the front by LAG_AV units, the
            # psum-coupled normalize trails by LAG_N so neither the Act nor
            # DVE queue head ever waits on a just-issued AV. outproj work is
            # spread one query-tile per unit; token-half A's reduce-scatter
            # runs under half B's attention.
            LAG_AV, LAG_CP, LAG_N = 6, 7, 9
            units = [(h, quarter * 4 + j)
                     for quarter in range(4) for h in range(HPC)
                     for j in range(4)]
            n_units = len(units)
            oproj_queue = []

            def step(idx):
                if pre_work:
                    pre_work.pop(0)()
                if idx < n_units:
                    front(*units[idx])
                if 0 <= idx - LAG_CP < n_units:
                    back_cp(*units[idx - LAG_CP])
                if 0 <= idx - LAG_AV < n_units:
                    back_av(*units[idx - LAG_AV])
                if 0 <= idx - LAG_N < n_units:
                    bh, bqt = units[idx - LAG_N]
                    back_norm(bh, bqt)
                    if bh == HPC - 1 and bqt % 4 == 3:
                        quarter = bqt // 4
                        aout_transpose(0, quarter)
                        aout_transpose(1, quarter)
                        oproj_queue.extend(
                            ("proj", (quarter * 4 + j, oc))
                            for j in range(4) for oc in range(2))
                        if quarter % 2 == 1:
                            oproj_queue.append(("rs", quarter // 2))
                if oproj_queue:
                    kind, arg = oproj_queue.pop(0)
                    if kind == "proj":
                        outproj_oc(*arg)
                    else:
                        reduce_scatter(arg)

            idx = 0
            while idx < n_units + LAG_N or oproj_queue:
                step(idx)
                idx += 1

    nc.finalize()
    return nc


_NC_CACHE = None


def kernel(x, mask, gamma, w_qkv, w_out):
    global _NC_CACHE
    x = np.asarray(x, dtype=np.float32)
    gamma = np.asarray(gamma, dtype=np.float32)
    w_qkv = np.asarray(w_qkv, dtype=np.float32)
    w_out = np.asarray(w_out, dtype=np.float32)

    # fold gamma (RMSNorm scale) and the x8 q-scale into w_qkv (exact in f32)
    w = w_qkv * gamma[:, None]
    w = np.concatenate([w[:, :D] * (DH ** 0.5), w[:, D:]], axis=1)

    if _NC_CACHE is None:
        _NC_CACHE = build_graph()
    nc = _NC_CACHE

    in_maps = []
    for c in range(NC_TOTAL):
        b, hg = divmod(c, GROUP)
        cs = slice(hg * HPC * DH, (hg + 1) * HPC * DH)
        wq = w[:, 0:D][:, cs]
        wk = w[:, D:2 * D][:, cs]
        wv = w[:, 2 * D:3 * D][:, cs]
        wc = np.ascontiguousarray(
            np.concatenate([wq, wk, wv], axis=1), dtype=np.float16)
        wo = np.ascontiguousarray(
            w_out[cs, :].astype(ml_dtypes.bfloat16))
        xs = np.ascontiguousarray(x[b], dtype=np.float16)
        in_maps.append({"x": xs, "w_qkv": wc, "w_out": wo})

    res = run_bass_kernel_spmd(nc, in_maps, core_ids=list(range(NC_TOTAL)))
    out = np.empty((B, N, D), dtype=np.float32)
    for c in range(NC_TOTAL):
        b, r = divmod(c, GROUP)
        o = np.asarray(res.results[c]["out"]).astype(np.float32)
        out[b, r * 256:(r + 1) * 256, :] = o[0:256]
        out[b, N // 2 + r * 256:N // 2 + (r + 1) * 256, :] = o[256:512]
    return out


# revision 36
# speedup vs baseline: 1.0110x; 1.0110x over previous
"""Distributed Bass kernel: RMSNorm + multi-head attention + out-proj on 8 TRN2 cores.

Sharding: head x batch tensor parallel. Core c owns batch c//4 and heads
[4*(c%4), 4*(c%4)+4) for the full 2048-token sequence. Each core RMSNorms the
whole batch, projects Q/K/V for only its 4 heads (w_qkv column shard), runs
full attention for those heads, and computes a partial output projection
(w_out row shard). A single bf16 ReduceScatter per token-half sums the 4
partials of each batch group and scatters 512 rows back to each core - the
only collective in the kernel (the baseline's 8 serialized K/V AllGathers
cost ~330us on the collective cores).

Attention pipeline per (head, 128-query tile): q-major sim on the PE
(fp16, x8 scale folded into w_q), exact row-max via DVE+Pool psum reduces,
one ScalarE exp pass (bias = -rowmax), DMA-xbar transpose of the bf16 attn
tile into keys-major layout, then a full-PE AV matmul (lhsT = attnT tile,
128x128 stationary; moving operand = [v | ones], 65 columns) whose extra
ones-column yields the softmax denominator for free. Normalization happens
on the tiny [128, 64] AV output, not the [128, 2048] attn matrix.
"""

import sys

sys.path.insert(0, "/opt/trn_rl_repo")

import numpy as np
import ml_dtypes

import concourse.bass as bass
import concourse.mybir as mybir
import concourse.tile as tile
from concourse import bacc
from concourse.bass_utils import run_bass_kernel_spmd
from concourse.masks import make_identity

F32 = mybir.dt.float32
F16 = mybir.dt.float16
BF16 = mybir.dt.bfloat16
AF = mybir.ActivationFunctionType
ALU = mybir.AluOpType

B, N, D = 2, 2048, 1024
H, DH = 16, 64
EPS = 1e-5
NC_TOTAL = 8
HPC = 4                 # heads per core
GROUP = 4               # cores per batch (reduce-scatter group)
NT = N // 128           # 16 token tiles
QT = NT                 # query tiles
KC = NT                 # key chunks of 128
DC = D // 128           # 8 contraction chunks
WQKV_COLS = 3 * HPC * DH  # 768


def build_graph():
    nc = bacc.Bacc(name="attn8")
    x_d = nc.dram_tensor("x", [N, D], F16, kind="ExternalInput")
    w_d = nc.dram_tensor("w_qkv", [D, WQKV_COLS], F16, kind="ExternalInput")
    wout_d = nc.dram_tensor("w_out", [HPC * DH, D], BF16, kind="ExternalInput")
    outp_d = nc.dram_tensor("outp", [N, D], BF16, kind="Internal")
    rsout_d = nc.dram_tensor("rsout", [N // GROUP, D], BF16, kind="Internal")
    out_d = nc.dram_tensor("out", [N // GROUP, D], BF16,
                           kind="ExternalOutput")  # [512, 1024]

    rg = [list(range(GROUP)), list(range(GROUP, 2 * GROUP))]

    with tile.TileContext(nc) as tc:
        with (
            tc.tile_pool(name="const", bufs=1) as constp,
            tc.tile_pool(name="xload", bufs=6) as xp,
            tc.tile_pool(name="xnorm", bufs=NT) as xnp,
            tc.tile_pool(name="xnT", bufs=DC) as xntp,
            tc.tile_pool(name="wqkv", bufs=DC) as wp,
            tc.tile_pool(name="wout", bufs=2) as woutp,
            tc.tile_pool(name="kq", bufs=2) as kqp,
            tc.tile_pool(name="vx", bufs=HPC) as vxp,
            tc.tile_pool(name="stats", bufs=8) as statsp,
            tc.tile_pool(name="scr", bufs=3) as scrp,
            tc.tile_pool(name="attn", bufs=6) as attnp,
            tc.tile_pool(name="attnT", bufs=8) as attntp,
            tc.tile_pool(name="aout", bufs=2 * QT) as aoutp,
            tc.tile_pool(name="aoutT", bufs=2) as aouttp,
            tc.tile_pool(name="osb", bufs=3) as osbp,
            tc.tile_pool(name="ps_a", bufs=7, space="PSUM") as psa,
            tc.tile_pool(name="ps_b", bufs=1, space="PSUM") as psb,
        ):
            identf = constp.tile([128, 128], F16, name="identf")
            make_identity(nc, identf[:])
            identb = constp.tile([128, 128], BF16, name="identb")
            make_identity(nc, identb[:])
            epsb = constp.tile([128, 1], F32, name="epsb")
            nc.any.memset(epsb[:], EPS)

            # ---------------- DMA loads ----------------
            xt = []
            for t in range(NT):
                xl = xp.tile([128, D], F16, name=f"x{t}", tag="x")
                nc.sync.dma_start(xl[:], x_d[t * 128:(t + 1) * 128, :])
                xt.append(xl)
            w_sb = []
            for dc in range(DC):
                w = wp.tile([128, WQKV_COLS], F16, name=f"w{dc}", tag="w")
                nc.sync.dma_start(w[:], w_d[dc * 128:(dc + 1) * 128, :])
                w_sb.append(w)
            wout_sb = []
            for i in range(2):
                w = woutp.tile([128, D], BF16, name=f"wo{i}", tag="wo")
                nc.sync.dma_start(w[:], wout_d[i * 128:(i + 1) * 128, :])
                wout_sb.append(w)

            # ---- RMSNorm + transpose + projections, half-interleaved so the
            # PE starts transposing/projecting while the second token half is
            # still normalizing.
            xn = [None] * NT
            xnT = []
            for dc in range(DC):
                xT = xntp.tile([128, N], F16, name=f"xnT{dc}", tag="xnT")
                xnT.append(xT)

            def norm_tile(t):
                scr = scrp.tile([128, D], F16, name=f"scr{t}", tag="scr")
                ssq = statsp.tile([128, 1], F32, name=f"ssq{t}", tag="ssq")
                nc.scalar.activation(scr[:], xt[t][:], AF.Square,
                                     accum_out=ssq[:])
                std = statsp.tile([128, 1], F32, name=f"std{t}", tag="ssq")
                nc.scalar.activation(std[:], ssq[:], AF.Sqrt, scale=1.0 / D,
                                     bias=epsb[:])
                rinv = statsp.tile([128, 1], F32, name=f"ri{t}", tag="ssq")
                nc.vector.reciprocal(rinv[:], std[:])
                x2 = xnp.tile([128, D], F16, name=f"xn{t}", tag="xn")
                nc.vector.tensor_scalar_mul(x2[:], xt[t][:], rinv[:])
                xn[t] = x2

            def xnT_half(dc, half):
                for sub in range(2):
                    tp = psa.tile([128, 512], F16, name=f"tp{dc}{half}{sub}",
                                  tag="sim")
                    for j in range(4):
                        t = half * 8 + sub * 4 + j
                        nc.tensor.transpose(
                            tp[:, j * 128:(j + 1) * 128],
                            xn[t][:, dc * 128:(dc + 1) * 128],
                            identf[:])
                    col = half * 1024 + sub * 512
                    nc.vector.tensor_copy(
                        xnT[dc][:, col:col + 512], tp[:])

            # kT/qT feature-major pair tiles [128 feats(2 heads), 2048 tok]
            kTp = [kqp.tile([128, N], F16, name=f"kT{i}", tag="kT", bufs=2)
                   for i in range(2)]
            qTp = [kqp.tile([128, N], F16, name=f"qT{i}", tag="qT", bufs=2)
                   for i in range(2)]

            def proj_half(pt, col0, i, half):
                for tc2 in range(2):
                    tcol = half * 1024 + tc2 * 512
                    ps = psa.tile([128, 512], F32, name=f"pp{col0}{i}{tcol}",
                                  tag="sim")
                    for dc in range(DC):
                        nc.tensor.matmul(
                            ps[:],
                            w_sb[dc][:, col0 + i * 128:col0 + (i + 1) * 128],
                            xnT[dc][:, tcol:tcol + 512],
                            start=(dc == 0), stop=(dc == DC - 1))
                    nc.vector.tensor_copy(pt[:, tcol:tcol + 512], ps[:])

            # v token-major, per head [128 k-part, 16 kc * 65] bf16 with a
            # ones column at slot 64 of each kc block (softmax denominator).
            vx = []
            for h in range(HPC):
                v = vxp.tile([128, KC * 65], BF16, name=f"vx{h}", tag="vx")
                nc.any.memset(
                    v[:].rearrange("p (kc c) -> p kc c", c=65)[:, :, 64:65],
                    1.0)
                vx.append(v)

            def v_proj(t):
                ps = psa.tile([128, 512], F32, name=f"pv{t}", tag="sim")
                for dc in range(DC):
                    nc.tensor.matmul(
                        ps[:, 0:HPC * DH],
                        xnT[dc][:, t * 128:(t + 1) * 128],
                        w_sb[dc][:, 2 * HPC * DH:3 * HPC * DH],
                        start=(dc == 0), stop=(dc == DC - 1))
                for h in range(HPC):
                    nc.vector.tensor_copy(
                        vx[h][:, t * 65:t * 65 + 64],
                        ps[:, h * 64:(h + 1) * 64])

            for t in range(8):
                norm_tile(t)
            for dc in range(DC):
                xnT_half(dc, 0)
            proj_half(kTp[0], HPC * DH, 0, 0)
            proj_half(qTp[0], 0, 0, 0)
            for t in range(8, NT):
                norm_tile(t)
            for dc in range(DC):
                xnT_half(dc, 1)
            proj_half(kTp[0], HPC * DH, 0, 1)
            proj_half(qTp[0], 0, 0, 1)
            for t in range(NT):
                v_proj(t)
            pre_work = [
                lambda h=half: proj_half(kTp[1], HPC * DH, 1, h)
                for half in range(2)
            ] + [
                lambda h=half: proj_half(qTp[1], 0, 1, h)
                for half in range(2)
            ]

            # ---------------- attention (software-pipelined) ----------------
            # Unit = (head, 128-query tile). front() runs sim -> max -> exp ->
            # DMA transpose; back() runs AV + normalize. back(u) is emitted
            # LAG units after front(u) so the in-order PE never stalls on the
            # cross-engine max/exp/transpose chain.
            aout_tiles = {}  # (hp, qt) -> [128 q, 128 f] bf16 pair tile
            aoutT = {0: None, 1: None}
            unit_state = {}

            def front(h, qt):
                i, row = h // 2, (h % 2) * 64
                sims = []
                sa = statsp.tile([128, 4], F32, name=f"sa{h}{qt}", tag="sa")
                for qtr in range(4):
                    ps = psa.tile([128, 512], F32, name=f"s{h}{qt}{qtr}",
                                  tag="sim")
                    nc.tensor.matmul(
                        ps[:],
                        qTp[i][row:row + 64, qt * 128:(qt + 1) * 128],
                        kTp[i][row:row + 64, qtr * 512:(qtr + 1) * 512],
                        start=True, stop=True)
                    nc.vector.tensor_reduce(
                        sa[:, qtr:qtr + 1], ps[:],
                        axis=mybir.AxisListType.X, op=ALU.max)
                    sims.append(ps)
                negm = statsp.tile([128, 1], F32, name=f"nm{h}{qt}", tag="nm")
                nc.vector.tensor_reduce(negm[:], sa[:],
                                        axis=mybir.AxisListType.X,
                                        op=ALU.max, negate=True)
                at = attnp.tile([128, N], BF16, name=f"at{h}{qt}", tag="at")
                for qtr in range(4):
                    nc.scalar.activation(
                        at[:, qtr * 512:(qtr + 1) * 512],
                        sims[qtr][:], AF.Exp, bias=negm[:])
                atT = attntp.tile([128, KC * 128], BF16, name=f"atT{h}{qt}",
                                  tag="atT")
                nc.sync.dma_start_transpose(
                    atT[:].rearrange("p (kc q) -> p kc q", q=128), at[:])
                unit_state[(h, qt)] = atT

            av_rot = {"tile": None, "n": 0}

            def back_av(h, qt):
                atT = unit_state.pop((h, qt))
                if av_rot["n"] == 0:
                    av_rot["tile"] = psb.tile([128, 512], F32,
                                              name=f"av{h}{qt}", tag="av")
                j = av_rot["n"]
                av_rot["n"] = (j + 1) % 7
                av = av_rot["tile"][:, j * 65:(j + 1) * 65]
                atT3 = atT[:].rearrange("p (kc q) -> p kc q", q=128)
                for kc in range(KC):
                    nc.tensor.matmul(
                        av,
                        atT3[:, kc, :],
                        vx[h][:, kc * 65:(kc + 1) * 65],
                        start=(kc == 0), stop=(kc == KC - 1))
                unit_state[("av", h, qt)] = av

            def back_cp(h, qt):
                av = unit_state.pop(("av", h, qt))
                avs = statsp.tile([128, 65], F32, name=f"avs{h}{qt}",
                                  tag="avs", bufs=4)
                nc.scalar.copy(avs[:], av)
                unit_state[("avs", h, qt)] = avs

            def back_norm(h, qt):
                avs = unit_state.pop(("avs", h, qt))
                rs = statsp.tile([128, 1], F32, name=f"rs{h}{qt}", tag="rs")
                nc.vector.reciprocal(rs[:], avs[:, 64:65])
                hp = h // 2
                if (hp, qt) not in aout_tiles:
                    aout_tiles[(hp, qt)] = aoutp.tile(
                        [128, 128], BF16, name=f"ao{hp}{qt}", tag="ao")
                # SBUF->SBUF per-q scale on the idle Pool engine
                nc.gpsimd.tensor_scalar_mul(
                    aout_tiles[(hp, qt)][:, (h % 2) * 64:(h % 2) * 64 + 64],
                    avs[:, 0:64], rs[:])

            def aout_transpose(hp, quarter):
                # transpose this quarter's aout pair tiles into aoutT[hp]
                if aoutT[hp] is None:
                    aoutT[hp] = aouttp.tile([128, N], BF16, name=f"aoT{hp}",
                                            tag="aT")
                aT = aoutT[hp]
                tp = psa.tile([128, 512], BF16, name=f"tpa{hp}{quarter}",
                              tag="sim")
                for j in range(4):
                    qt = quarter * 4 + j
                    nc.tensor.transpose(
                        tp[:, j * 128:(j + 1) * 128],
                        aout_tiles[(hp, qt)][:], identb[:])
                col = quarter * 512
                nc.vector.tensor_copy(aT[:, col:col + 512], tp[:])

            oproj_tiles = {}

            def outproj_oc(qt, oc):
                if qt not in oproj_tiles:
                    oproj_tiles[qt] = osbp.tile([128, D], BF16, name=f"o{qt}",
                                                tag="o")
                ot = oproj_tiles[qt]
                ps = psa.tile([128, 512], F32, name=f"po{qt}{oc}", tag="sim")
                for hp in range(2):
                    nc.tensor.matmul(
                        ps[:],
                        aoutT[hp][:, qt * 128:(qt + 1) * 128],
                        wout_sb[hp][:, oc * 512:(oc + 1) * 512],
                        start=(hp == 0), stop=(hp == 1))
                if oc == 0:
                    nc.vector.tensor_copy(ot[:, 0:512], ps[:])
                else:
                    nc.scalar.copy(ot[:, 512:1024], ps[:])
                if oc == 1:
                    # Pool SWDGE queue so output DMAs never block the attn
                    # transposes queued on SP
                    nc.gpsimd.dma_start(outp_d[qt * 128:(qt + 1) * 128, :],
                                        ot[:])

            def reduce_scatter(half):
                import os as _os
                if _os.environ.get("KERNEL_FAKE_COMM") == "1":
                    nc.sync.dma_start(
                        rsout_d[half * 256:(half + 1) * 256, :],
                        outp_d[half * 1024:half * 1024 + 256, :])
                else:
                    nc.gpsimd.collective_compute(
                        "ReduceScatter", ALU.add, replica_groups=rg,
                        ins=[outp_d[half * 1024:(half + 1) * 1024, :].opt()],
                        outs=[rsout_d[half * 256:(half + 1) * 256, :].opt()])
                nc.sync.dma_start(out_d[half * 256:(half + 1) * 256, :],
                                  rsout_d[half * 256:(half + 1) * 256, :])

            # Software pipeline: AV trails the front by LAG_AV units, the
            # psum-coupled normalize trails by LAG_N so neither the Act nor
            # DVE queue head ever waits on a just-issued AV. outproj work is
            # spread one query-tile per unit; token-half A's reduce-scatter
            # runs under half B's attention.
            LAG_AV, LAG_CP, LAG_N = 5, 6, 8
            units = [(h, quarter * 4 + j)
                     for quarter in range(4) for h in range(HPC)
                     for j in range(4)]
            n_units = len(units)
            oproj_queue = []

            def step(idx):
                if pre_work:
                    pre_work.pop(0)()
                if idx < n_units:
                    front(*units[idx])
                if 0 <= idx - LAG_CP < n_units:
                    back_cp(*units[idx - LAG_CP])
                if 0 <= idx - LAG_AV < n_units:
                    back_av(*units[idx - LAG_AV])
                if 0 <= idx - LAG_N < n_units:
                    bh, bqt = units[idx - LAG_N]
                    back_norm(bh, bqt)
                    if bh == HPC - 1 and bqt % 4 == 3:
                        quarter = bqt // 4
                        aout_transpose(0, quarter)
                        aout_transpose(1, quarter)
                        oproj_queue.extend(
                            ("proj", (quarter * 4 + j, oc))
                            for j in range(4) for oc in range(2))
                        if quarter % 2 == 1:
                            oproj_queue.append(("rs", quarter // 2))
                if oproj_queue:
                    kind, arg = oproj_queue.pop(0)
                    if kind == "proj":
                        outproj_oc(*arg)
                    else:
                        reduce_scatter(arg)

            idx = 0
            while idx < n_units + LAG_N or oproj_queue:
                step(idx)
                idx += 1

    nc.finalize()
    return nc


_NC_CACHE = None


def kernel(x, mask, gamma, w_qkv, w_out):
    global _NC_CACHE
    x = np.asarray(x, dtype=np.float32)
    gamma = np.asarray(gamma, dtype=np.float32)
    w_qkv = np.asarray(w_qkv, dtype=np.float32)
    w_out = np.asarray(w_out, dtype=np.float32)

    # fold gamma (RMSNorm scale) and the x8 q-scale into w_qkv (exact in f32)
    w = w_qkv * gamma[:, None]
    w = np.concatenate([w[:, :D] * (DH ** 0.5), w[:, D:]], axis=1)

    if _NC_CACHE is None:
        _NC_CACHE = build_graph()
    nc = _NC_CACHE

    in_maps = []
    for c in range(NC_TOTAL):
        b, hg = divmod(c, GROUP)
        cs = slice(hg * HPC * DH, (hg + 1) * HPC * DH)
        wq = w[:, 0:D][:, cs]
        wk = w[:, D:2 * D][:, cs]
        wv = w[:, 2 * D:3 * D][:, cs]
        wc = np.ascontiguousarray(
            np.concatenate([wq, wk, wv], axis=1), dtype=np.float16)
        wo = np.ascontiguousarray(
            w_out[cs, :].astype(ml_dtypes.bfloat16))
        xs = np.ascontiguousarray(x[b], dtype=np.float16)
        in_maps.append({"x": xs, "w_qkv": wc, "w_out": wo})

    res = run_bass_kernel_spmd(nc, in_maps, core_ids=list(range(NC_TOTAL)))
    out = np.empty((B, N, D), dtype=np.float32)
    for c in range(NC_TOTAL):
        b, r = divmod(c, GROUP)
        o = np.asarray(res.results[c]["out"]).astype(np.float32)
        out[b, r * 256:(r + 1) * 256, :] = o[0:256]
        out[b, N // 2 + r * 256:N // 2 + (r + 1) * 256, :] = o[256:512]
    return out


# revision 37
# speedup vs baseline: 1.0144x; 1.0034x over previous
"""Distributed Bass kernel: RMSNorm + multi-head attention + out-proj on 8 TRN2 cores.

Sharding: head x batch tensor parallel. Core c owns batch c//4 and heads
[4*(c%4), 4*(c%4)+4) for the full 2048-token sequence. Each core RMSNorms the
whole batch, projects Q/K/V for only its 4 heads (w_qkv column shard), runs
full attention for those heads, and computes a partial output projection
(w_out row shard). A single bf16 ReduceScatter per token-half sums the 4
partials of each batch group and scatters 512 rows back to each core - the
only collective in the kernel (the baseline's 8 serialized K/V AllGathers
cost ~330us on the collective cores).

Attention pipeline per (head, 128-query tile): q-major sim on the PE
(fp16, x8 scale folded into w_q), exact row-max via DVE+Pool psum reduces,
one ScalarE exp pass (bias = -rowmax), DMA-xbar transpose of the bf16 attn
tile into keys-major layout, then a full-PE AV matmul (lhsT = attnT tile,
128x128 stationary; moving operand = [v | ones], 65 columns) whose extra
ones-column yields the softmax denominator for free. Normalization happens
on the tiny [128, 64] AV output, not the [128, 2048] attn matrix.
"""

import sys

sys.path.insert(0, "/opt/trn_rl_repo")

import numpy as np
import ml_dtypes

import concourse.bass as bass
import concourse.mybir as mybir
import concourse.tile as tile
from concourse import bacc
from concourse.bass_utils import run_bass_kernel_spmd
from concourse.masks import make_identity

F32 = mybir.dt.float32
F16 = mybir.dt.float16
BF16 = mybir.dt.bfloat16
AF = mybir.ActivationFunctionType
ALU = mybir.AluOpType

B, N, D = 2, 2048, 1024
H, DH = 16, 64
EPS = 1e-5
NC_TOTAL = 8
HPC = 4                 # heads per core
GROUP = 4               # cores per batch (reduce-scatter group)
NT = N // 128           # 16 token tiles
QT = NT                 # query tiles
KC = NT                 # key chunks of 128
DC = D // 128           # 8 contraction chunks
WQKV_COLS = 3 * HPC * DH  # 768


def build_graph():
    nc = bacc.Bacc(name="attn8")
    x_d = nc.dram_tensor("x", [N, D], F16, kind="ExternalInput")
    w_d = nc.dram_tensor("w_qkv", [D, WQKV_COLS], F16, kind="ExternalInput")
    wout_d = nc.dram_tensor("w_out", [HPC * DH, D], BF16, kind="ExternalInput")
    outp_d = nc.dram_tensor("outp", [N, D], BF16, kind="Internal")
    rsout_d = nc.dram_tensor("rsout", [N // GROUP, D], BF16, kind="Internal")
    out_d = nc.dram_tensor("out", [N // GROUP, D], BF16,
                           kind="ExternalOutput")  # [512, 1024]

    rg = [list(range(GROUP)), list(range(GROUP, 2 * GROUP))]

    with tile.TileContext(nc) as tc:
        with (
            tc.tile_pool(name="const", bufs=1) as constp,
            tc.tile_pool(name="xload", bufs=6) as xp,
            tc.tile_pool(name="xnorm", bufs=NT) as xnp,
            tc.tile_pool(name="xnT", bufs=DC) as xntp,
            tc.tile_pool(name="wqkv", bufs=DC) as wp,
            tc.tile_pool(name="wout", bufs=2) as woutp,
            tc.tile_pool(name="kq", bufs=2) as kqp,
            tc.tile_pool(name="vx", bufs=HPC) as vxp,
            tc.tile_pool(name="stats", bufs=12) as statsp,
            tc.tile_pool(name="scr", bufs=3) as scrp,
            tc.tile_pool(name="attn", bufs=6) as attnp,
            tc.tile_pool(name="attnT", bufs=8) as attntp,
            tc.tile_pool(name="aout", bufs=2 * QT) as aoutp,
            tc.tile_pool(name="aoutT", bufs=2) as aouttp,
            tc.tile_pool(name="osb", bufs=4) as osbp,
            tc.tile_pool(name="ps_a", bufs=7, space="PSUM") as psa,
            tc.tile_pool(name="ps_b", bufs=1, space="PSUM") as psb,
        ):
            identf = constp.tile([128, 128], F16, name="identf")
            make_identity(nc, identf[:])
            identb = constp.tile([128, 128], BF16, name="identb")
            make_identity(nc, identb[:])
            epsb = constp.tile([128, 1], F32, name="epsb")
            nc.any.memset(epsb[:], EPS)

            # ---------------- DMA loads ----------------
            xt = []
            for t in range(NT):
                xl = xp.tile([128, D], F16, name=f"x{t}", tag="x")
                nc.sync.dma_start(xl[:], x_d[t * 128:(t + 1) * 128, :])
                xt.append(xl)
            w_sb = []
            for dc in range(DC):
                w = wp.tile([128, WQKV_COLS], F16, name=f"w{dc}", tag="w")
                nc.sync.dma_start(w[:], w_d[dc * 128:(dc + 1) * 128, :])
                w_sb.append(w)
            wout_sb = []
            for i in range(2):
                w = woutp.tile([128, D], BF16, name=f"wo{i}", tag="wo")
                nc.sync.dma_start(w[:], wout_d[i * 128:(i + 1) * 128, :])
                wout_sb.append(w)

            # ---- RMSNorm + transpose + projections, half-interleaved so the
            # PE starts transposing/projecting while the second token half is
            # still normalizing.
            xn = [None] * NT
            xnT = []
            for dc in range(DC):
                xT = xntp.tile([128, N], F16, name=f"xnT{dc}", tag="xnT")
                xnT.append(xT)

            def norm_tile(t):
                scr = scrp.tile([128, D], F16, name=f"scr{t}", tag="scr")
                ssq = statsp.tile([128, 1], F32, name=f"ssq{t}", tag="ssq")
                nc.scalar.activation(scr[:], xt[t][:], AF.Square,
                                     accum_out=ssq[:])
                std = statsp.tile([128, 1], F32, name=f"std{t}", tag="ssq")
                nc.scalar.activation(std[:], ssq[:], AF.Sqrt, scale=1.0 / D,
                                     bias=epsb[:])
                rinv = statsp.tile([128, 1], F32, name=f"ri{t}", tag="ssq")
                nc.vector.reciprocal(rinv[:], std[:])
                x2 = xnp.tile([128, D], F16, name=f"xn{t}", tag="xn")
                nc.vector.tensor_scalar_mul(x2[:], xt[t][:], rinv[:])
                xn[t] = x2

            def xnT_half(dc, half):
                for sub in range(2):
                    tp = psa.tile([128, 512], F16, name=f"tp{dc}{half}{sub}",
                                  tag="sim")
                    for j in range(4):
                        t = half * 8 + sub * 4 + j
                        nc.tensor.transpose(
                            tp[:, j * 128:(j + 1) * 128],
                            xn[t][:, dc * 128:(dc + 1) * 128],
                            identf[:])
                    col = half * 1024 + sub * 512
                    nc.vector.tensor_copy(
                        xnT[dc][:, col:col + 512], tp[:])

            # kT/qT feature-major pair tiles [128 feats(2 heads), 2048 tok]
            kTp = [kqp.tile([128, N], F16, name=f"kT{i}", tag="kT", bufs=2)
                   for i in range(2)]
            qTp = [kqp.tile([128, N], F16, name=f"qT{i}", tag="qT", bufs=2)
                   for i in range(2)]

            def proj_half(pt, col0, i, half):
                for tc2 in range(2):
                    tcol = half * 1024 + tc2 * 512
                    ps = psa.tile([128, 512], F32, name=f"pp{col0}{i}{tcol}",
                                  tag="sim")
                    for dc in range(DC):
                        nc.tensor.matmul(
                            ps[:],
                            w_sb[dc][:, col0 + i * 128:col0 + (i + 1) * 128],
                            xnT[dc][:, tcol:tcol + 512],
                            start=(dc == 0), stop=(dc == DC - 1))
                    nc.vector.tensor_copy(pt[:, tcol:tcol + 512], ps[:])

            # v token-major, per head [128 k-part, 16 kc * 65] bf16 with a
            # ones column at slot 64 of each kc block (softmax denominator).
            vx = []
            for h in range(HPC):
                v = vxp.tile([128, KC * 65], BF16, name=f"vx{h}", tag="vx")
                nc.any.memset(
                    v[:].rearrange("p (kc c) -> p kc c", c=65)[:, :, 64:65],
                    1.0)
                vx.append(v)

            def v_proj(t):
                ps = psa.tile([128, 512], F32, name=f"pv{t}", tag="sim")
                for dc in range(DC):
                    nc.tensor.matmul(
                        ps[:, 0:HPC * DH],
                        xnT[dc][:, t * 128:(t + 1) * 128],
                        w_sb[dc][:, 2 * HPC * DH:3 * HPC * DH],
                        start=(dc == 0), stop=(dc == DC - 1))
                for h in range(HPC):
                    nc.vector.tensor_copy(
                        vx[h][:, t * 65:t * 65 + 64],
                        ps[:, h * 64:(h + 1) * 64])

            for t in range(8):
                norm_tile(t)
            for dc in range(DC):
                xnT_half(dc, 0)
            proj_half(kTp[0], HPC * DH, 0, 0)
            proj_half(qTp[0], 0, 0, 0)
            for t in range(8, NT):
                norm_tile(t)
            for dc in range(DC):
                xnT_half(dc, 1)
            proj_half(kTp[0], HPC * DH, 0, 1)
            proj_half(qTp[0], 0, 0, 1)
            for t in range(NT):
                v_proj(t)
            pre_work = [
                lambda h=half: proj_half(kTp[1], HPC * DH, 1, h)
                for half in range(2)
            ] + [
                lambda h=half: proj_half(qTp[1], 0, 1, h)
                for half in range(2)
            ]

            # ---------------- attention (software-pipelined) ----------------
            # Unit = (head, 128-query tile). front() runs sim -> max -> exp ->
            # DMA transpose; back() runs AV + normalize. back(u) is emitted
            # LAG units after front(u) so the in-order PE never stalls on the
            # cross-engine max/exp/transpose chain.
            aout_tiles = {}  # (hp, qt) -> [128 q, 128 f] bf16 pair tile
            aoutT = {0: None, 1: None}
            unit_state = {}

            def front(h, qt):
                i, row = h // 2, (h % 2) * 64
                sims = []
                sa = statsp.tile([128, 4], F32, name=f"sa{h}{qt}", tag="sa")
                for qtr in range(4):
                    ps = psa.tile([128, 512], F32, name=f"s{h}{qt}{qtr}",
                                  tag="sim")
                    nc.tensor.matmul(
                        ps[:],
                        qTp[i][row:row + 64, qt * 128:(qt + 1) * 128],
                        kTp[i][row:row + 64, qtr * 512:(qtr + 1) * 512],
                        start=True, stop=True)
                    nc.vector.tensor_reduce(
                        sa[:, qtr:qtr + 1], ps[:],
                        axis=mybir.AxisListType.X, op=ALU.max)
                    sims.append(ps)
                negm = statsp.tile([128, 1], F32, name=f"nm{h}{qt}", tag="nm")
                nc.vector.tensor_reduce(negm[:], sa[:],
                                        axis=mybir.AxisListType.X,
                                        op=ALU.max, negate=True)
                at = attnp.tile([128, N], BF16, name=f"at{h}{qt}", tag="at")
                for qtr in range(4):
                    nc.scalar.activation(
                        at[:, qtr * 512:(qtr + 1) * 512],
                        sims[qtr][:], AF.Exp, bias=negm[:])
                atT = attntp.tile([128, KC * 128], BF16, name=f"atT{h}{qt}",
                                  tag="atT")
                nc.sync.dma_start_transpose(
                    atT[:].rearrange("p (kc q) -> p kc q", q=128), at[:])
                unit_state[(h, qt)] = atT

            av_rot = {"tile": None, "n": 0}

            def back_av(h, qt):
                atT = unit_state.pop((h, qt))
                if av_rot["n"] == 0:
                    av_rot["tile"] = psb.tile([128, 512], F32,
                                              name=f"av{h}{qt}", tag="av")
                j = av_rot["n"]
                av_rot["n"] = (j + 1) % 7
                av = av_rot["tile"][:, j * 65:(j + 1) * 65]
                atT3 = atT[:].rearrange("p (kc q) -> p kc q", q=128)
                for kc in range(KC):
                    nc.tensor.matmul(
                        av,
                        atT3[:, kc, :],
                        vx[h][:, kc * 65:(kc + 1) * 65],
                        start=(kc == 0), stop=(kc == KC - 1))
                unit_state[("av", h, qt)] = av

            def back_cp(h, qt):
                av = unit_state.pop(("av", h, qt))
                avs = statsp.tile([128, 65], F32, name=f"avs{h}{qt}",
                                  tag="avs", bufs=4)
                nc.scalar.copy(avs[:], av)
                unit_state[("avs", h, qt)] = avs

            def back_norm(h, qt):
                avs = unit_state.pop(("avs", h, qt))
                rs = statsp.tile([128, 1], F32, name=f"rs{h}{qt}", tag="rs")
                nc.vector.reciprocal(rs[:], avs[:, 64:65])
                hp = h // 2
                if (hp, qt) not in aout_tiles:
                    aout_tiles[(hp, qt)] = aoutp.tile(
                        [128, 128], BF16, name=f"ao{hp}{qt}", tag="ao")
                # SBUF->SBUF per-q scale on the idle Pool engine
                nc.gpsimd.tensor_scalar_mul(
                    aout_tiles[(hp, qt)][:, (h % 2) * 64:(h % 2) * 64 + 64],
                    avs[:, 0:64], rs[:])

            def aout_transpose(hp, quarter):
                # transpose this quarter's aout pair tiles into aoutT[hp]
                if aoutT[hp] is None:
                    aoutT[hp] = aouttp.tile([128, N], BF16, name=f"aoT{hp}",
                                            tag="aT")
                aT = aoutT[hp]
                tp = psa.tile([128, 512], BF16, name=f"tpa{hp}{quarter}",
                              tag="sim")
                for j in range(4):
                    qt = quarter * 4 + j
                    nc.tensor.transpose(
                        tp[:, j * 128:(j + 1) * 128],
                        aout_tiles[(hp, qt)][:], identb[:])
                col = quarter * 512
                nc.vector.tensor_copy(aT[:, col:col + 512], tp[:])

            oproj_tiles = {}

            def outproj_oc(qt, oc):
                if qt not in oproj_tiles:
                    oproj_tiles[qt] = osbp.tile([128, D], BF16, name=f"o{qt}",
                                                tag="o")
                ot = oproj_tiles[qt]
                ps = psa.tile([128, 512], F32, name=f"po{qt}{oc}", tag="sim")
                for hp in range(2):
                    nc.tensor.matmul(
                        ps[:],
                        aoutT[hp][:, qt * 128:(qt + 1) * 128],
                        wout_sb[hp][:, oc * 512:(oc + 1) * 512],
                        start=(hp == 0), stop=(hp == 1))
                if oc == 0:
                    nc.vector.tensor_copy(ot[:, 0:512], ps[:])
                else:
                    nc.scalar.copy(ot[:, 512:1024], ps[:])
                if oc == 1:
                    # Pool SWDGE queue so output DMAs never block the attn
                    # transposes queued on SP
                    nc.gpsimd.dma_start(outp_d[qt * 128:(qt + 1) * 128, :],
                                        ot[:])

            def reduce_scatter(half):
                import os as _os
                if _os.environ.get("KERNEL_FAKE_COMM") == "1":
                    nc.sync.dma_start(
                        rsout_d[half * 256:(half + 1) * 256, :],
                        outp_d[half * 1024:half * 1024 + 256, :])
                else:
                    nc.gpsimd.collective_compute(
                        "ReduceScatter", ALU.add, replica_groups=rg,
                        ins=[outp_d[half * 1024:(half + 1) * 1024, :].opt()],
                        outs=[rsout_d[half * 256:(half + 1) * 256, :].opt()])
                nc.sync.dma_start(out_d[half * 256:(half + 1) * 256, :],
                                  rsout_d[half * 256:(half + 1) * 256, :])

            # Software pipeline: AV trails the front by LAG_AV units, the
            # psum-coupled normalize trails by LAG_N so neither the Act nor
            # DVE queue head ever waits on a just-issued AV. outproj work is
            # spread one query-tile per unit; token-half A's reduce-scatter
            # runs under half B's attention.
            LAG_AV, LAG_CP, LAG_N = 5, 6, 8
            units = [(h, quarter * 4 + j)
                     for quarter in range(4) for h in range(HPC)
                     for j in range(4)]
            n_units = len(units)
            oproj_queue = []

            def step(idx):
                if pre_work:
                    pre_work.pop(0)()
                if idx < n_units:
                    front(*units[idx])
                if 0 <= idx - LAG_CP < n_units:
                    back_cp(*units[idx - LAG_CP])
                if 0 <= idx - LAG_AV < n_units:
                    back_av(*units[idx - LAG_AV])
                if 0 <= idx - LAG_N < n_units:
                    bh, bqt = units[idx - LAG_N]
                    back_norm(bh, bqt)
                    if bh == HPC - 1 and bqt % 4 == 3:
                        quarter = bqt // 4
                        aout_transpose(0, quarter)
                        aout_transpose(1, quarter)
                        oproj_queue.extend(
                            ("proj", (quarter * 4 + j, oc))
                            for j in range(4) for oc in range(2))
                        if quarter % 2 == 1:
                            oproj_queue.append(("rs", quarter // 2))
                if oproj_queue:
                    kind, arg = oproj_queue.pop(0)
                    if kind == "proj":
                        outproj_oc(*arg)
                    else:
                        reduce_scatter(arg)

            idx = 0
            while idx < n_units + LAG_N or oproj_queue:
                step(idx)
                idx += 1

    nc.finalize()
    return nc


_NC_CACHE = None


def kernel(x, mask, gamma, w_qkv, w_out):
    global _NC_CACHE
    x = np.asarray(x, dtype=np.float32)
    gamma = np.asarray(gamma, dtype=np.float32)
    w_qkv = np.asarray(w_qkv, dtype=np.float32)
    w_out = np.asarray(w_out, dtype=np.float32)

    # fold gamma (RMSNorm scale) and the x8 q-scale into w_qkv (exact in f32)
    w = w_qkv * gamma[:, None]
    w = np.concatenate([w[:, :D] * (DH ** 0.5), w[:, D:]], axis=1)

    if _NC_CACHE is None:
        _NC_CACHE = build_graph()
    nc = _NC_CACHE

    in_maps = []
    for c in range(NC_TOTAL):
        b, hg = divmod(c, GROUP)
        cs = slice(hg * HPC * DH, (hg + 1) * HPC * DH)
        wq = w[:, 0:D][:, cs]
        wk = w[:, D:2 * D][:, cs]
        wv = w[:, 2 * D:3 * D][:, cs]
        wc = np.ascontiguousarray(
            np.concatenate([wq, wk, wv], axis=1), dtype=np.float16)
        wo = np.ascontiguousarray(
            w_out[cs, :].astype(ml_dtypes.bfloat16))
        xs = np.ascontiguousarray(x[b], dtype=np.float16)
        in_maps.append({"x": xs, "w_qkv": wc, "w_out": wo})

    res = run_bass_kernel_spmd(nc, in_maps, core_ids=list(range(NC_TOTAL)))
    out = np.empty((B, N, D), dtype=np.float32)
    for c in range(NC_TOTAL):
        b, r = divmod(c, GROUP)
        o = np.asarray(res.results[c]["out"]).astype(np.float32)
        out[b, r * 256:(r + 1) * 256, :] = o[0:256]
        out[b, N // 2 + r * 256:N // 2 + (r + 1) * 256, :] = o[256:512]
    return out
